# revision 3
# baseline (speedup 1.0000x reference)
"""Bass/Trainium2 kernel for nn_ExpMovAvgModel (sparse_attention).

Math (per batch row b, query t, key s, H=128 hidden):
    x      = embd[seq]                        # [T, H] gathered rows
    xhat   = x / |x|                          # row-normalized
    raw    = xhat @ xhat.T                    # cosine similarity [T, T]
    sim01  = 0.5*(raw+1) masked to s < t
    delta  = reversed-cumsum_s(sim01)         # = sum_{v=s}^{t-1} sim01[v]
    lam    = exp(x @ lam_w + lam_b)
    w      = sim01 * exp(-lam*delta)
    yhat   = clip((w @ y) / (sum_s w + 1e-6), 0.01, 0.99)

Key restructure used here: with q[u] = exp(-lam*sim01[u]) and
d1[u] = (raw[u]+1)*q[u] = 2*sim01[u]*q[u], the forward scan
    S[s] = q[s]*S[s-1] + d1[s]
satisfies S[t-1] = 2*sum_u sim01[u]*exp(-lam*sum_{v=u}^{t-1} sim01[v])
        = 2*sum_s w[t,s].
So one tensor_tensor_scan per row-block + a diagonal extraction replaces
the masked reversed cumsum, the exp over [T,T], the weight multiply and
the row reduction.  A second scan with d1y = d1*y gives 2*(w @ y).
Causality comes for free: positions u >= t never enter S[t-1].

Sharding: data-parallel over batch B=32 -> 4 batches per core x 8 cores.
"""

import os
import sys

import numpy as np

for _p in ("/opt/trn_rl_repo",):
    if _p not in sys.path and os.path.isdir(_p):
        sys.path.append(_p)

import concourse.bass as bass
import concourse.tile as tile
from concourse import bacc, mybir
from concourse.bass_utils import run_bass_kernel_spmd

P = 128            # partitions / hidden dim
T = 1024           # sequence length
NJ = T // P        # 8 column-blocks
NB_PER_CORE = 4    # batches per core
N_CORES = 8
N_VOCAB = 50000

F32 = mybir.dt.float32
I32 = mybir.dt.int32

# engine knobs (tuned against the profile)
D1Y_ENGINE = "gpsimd"     # d1y = d1 * y elementwise multiply
EXTRACT_ENGINE = "vector" # diagonal extraction of the scan outputs (Pool lacks TensorScalarPtr)
SCAN_Y_ENGINE = "vector"  # second scan


def _eng(nc, name):
    return getattr(nc, name)


def build_program():
    nc = bacc.Bacc(
        "TRN2",
        target_bir_lowering=False,
        debug=False,
        num_devices=N_CORES,
    )

    table = nc.dram_tensor("table", [N_VOCAB, P], F32, kind="ExternalInput").ap()
    idx = nc.dram_tensor("idx", [NB_PER_CORE, P, NJ], I32, kind="ExternalInput").ap()
    ybc = nc.dram_tensor("ybc", [NB_PER_CORE, P, T], F32, kind="ExternalInput").ap()
    lamw = nc.dram_tensor("lamw", [P, 1], F32, kind="ExternalInput").ap()
    lamb = nc.dram_tensor("lamb", [P, 1], F32, kind="ExternalInput").ap()
    diag = nc.dram_tensor("diag", [P, P], F32, kind="ExternalInput").ap()
    out = nc.dram_tensor("out", [NB_PER_CORE, P, NJ], F32, kind="ExternalOutput").ap()

    with tile.TileContext(nc) as tc:
        _build_body(tc, table, idx, ybc, lamw, lamb, diag, out)

    nc.compile()
    return nc


def _build_body(tc, table, idx, ybc, lamw, lamb, diag, out):
    from contextlib import ExitStack

    nc = tc.nc
    Exp = mybir.ActivationFunctionType.Exp
    Sqrt = mybir.ActivationFunctionType.Sqrt
    ADD = mybir.AluOpType.add
    MULT = mybir.AluOpType.mult
    MAX = mybir.AluOpType.max
    MIN = mybir.AluOpType.min

    with ExitStack() as ctx:
        pconst = ctx.enter_context(tc.tile_pool(name="pconst", bufs=1))
        pbatch = ctx.enter_context(tc.tile_pool(name="pbatch", bufs=2))
        psmall = ctx.enter_context(tc.tile_pool(name="psmall", bufs=2))
        pmain = ctx.enter_context(tc.tile_pool(name="pmain", bufs=2))
        pps = ctx.enter_context(tc.tile_pool(name="pps", bufs=4, space="PSUM"))
        ppsx = ctx.enter_context(tc.tile_pool(name="ppsx", bufs=2, space="PSUM"))
        ppsl = ctx.enter_context(tc.tile_pool(name="ppsl", bufs=1, space="PSUM"))

        diag_sb = pconst.tile([P, P], F32)
        nc.sync.dma_start(out=diag_sb[:], in_=diag)
        lamw_sb = pconst.tile([P, 1], F32)
        nc.sync.dma_start(out=lamw_sb[:], in_=lamw)
        lamb_sb = pconst.tile([P, 1], F32)
        nc.sync.dma_start(out=lamb_sb[:], in_=lamb)

        for b in range(NB_PER_CORE):
            # ---- gather x rows: xg_j[p, :] = table[seq[j*128+p]] ----
            # HW indirect DMA needs one index per partition and a
            # contiguous output tile, so gather per column-block.
            idx_sb = psmall.tile([P, NJ], I32, tag="idx_sb")
            nc.sync.dma_start(out=idx_sb[:], in_=idx[b])
            xgs = []
            for j in range(NJ):
                xg = pbatch.tile([P, P], F32, tag=f"xg{j}")
                nc.gpsimd.indirect_dma_start(
                    out=xg[:],
                    out_offset=None,
                    in_=table,
                    in_offset=bass.IndirectOffsetOnAxis(
                        ap=idx_sb[:, j : j + 1], axis=0
                    ),
                )
                xgs.append(xg)

            # ---- row norms -> mag, rmag; normalize x ----
            magsq = psmall.tile([P, NJ], F32, tag="magsq")
            for j in range(NJ):
                sqjunk = pmain.tile([P, P], F32, tag="sqjunk")
                nc.scalar.activation(
                    out=sqjunk[:],
                    in_=xgs[j][:],
                    func=mybir.ActivationFunctionType.Square,
                    accum_out=magsq[:, j : j + 1],
                )
            mag = psmall.tile([P, NJ], F32, tag="mag")
            nc.scalar.activation(out=mag[:], in_=magsq[:], func=Sqrt)
            rmag = psmall.tile([P, NJ], F32, tag="rmag")
            nc.vector.reciprocal(out=rmag[:], in_=mag[:])
            xhat = pbatch.tile([P, NJ, P], F32, tag="xhat")
            for j in range(NJ):
                nc.vector.tensor_scalar(
                    out=xhat[:, j, :],
                    in0=xgs[j][:],
                    scalar1=rmag[:, j : j + 1],
                    scalar2=None,
                    op0=MULT,
                )

            # ---- transpose xhat -> xhatT [h, t] via PE ----
            xhatT = pbatch.tile([P, T], F32, tag="xhatT")
            for half in range(2):
                xt_ps = ppsx.tile([P, 512], F32, tag="xt_ps")
                for k in range(4):
                    j = half * 4 + k
                    nc.tensor.transpose(
                        out=xt_ps[:, k * P : (k + 1) * P],
                        in_=xhat[:, j, :],
                        identity=diag_sb[:],
                    )
                nc.scalar.copy(
                    out=xhatT[:, half * 512 : (half + 1) * 512], in_=xt_ps[:]
                )

            # ---- lam = exp(mag * (xhat . lam_w) + lam_b); nhl = -lam/2 ----
            lamdot_ps = ppsl.tile([P, NJ], F32, tag="lamdot_ps")
            for j in range(NJ):
                nc.tensor.matmul(
                    out=lamdot_ps[:, j : j + 1],
                    lhsT=xhatT[:, j * P : (j + 1) * P],
                    rhs=lamw_sb[:],
                    start=True,
                    stop=True,
                )
            lam = psmall.tile([P, NJ], F32, tag="lam")
            for j in range(NJ):
                nc.scalar.activation(
                    out=lam[:, j : j + 1],
                    in_=lamdot_ps[:, j : j + 1],
                    func=Exp,
                    bias=lamb_sb[:],
                    scale=mag[:, j : j + 1],
                )
            nhl = psmall.tile([P, NJ], F32, tag="nhl")
            nc.vector.tensor_scalar(
                out=nhl[:], in0=lam[:], scalar1=-0.5, scalar2=None, op0=MULT
            )

            # ---- y broadcast rows ----
            ybc_sb = pbatch.tile([P, T], F32, tag="ybc_sb")
            nc.sync.dma_start(out=ybc_sb[:], in_=ybc[b])

            wsum = psmall.tile([P, NJ], F32, tag="wsum")
            ynum = psmall.tile([P, NJ], F32, tag="ynum")

            # ---- main loop over query blocks ----
            for tb in range(NJ):
                W = (tb + 1) * P
                nhalf = 1 if W <= 512 else 2
                q = pmain.tile([P, T], F32, tag="q")
                d1 = pmain.tile([P, T], F32, tag="d1")
                d1y = pmain.tile([P, T], F32, tag="d1y")
                for h in range(nhalf):
                    w0 = h * 512
                    wh = min(W, (h + 1) * 512) - w0
                    raw_ps = pps.tile([P, 512], F32, tag="raw_ps")
                    nc.tensor.matmul(
                        out=raw_ps[:, :wh],
                        lhsT=xhatT[:, tb * P : (tb + 1) * P],
                        rhs=xhatT[:, w0 : w0 + wh],
                        start=True,
                        stop=True,
                    )
                    # q = exp(-lam/2 * raw - lam/2)
                    nc.scalar.activation(
                        out=q[:, w0 : w0 + wh],
                        in_=raw_ps[:, :wh],
                        func=Exp,
                        bias=nhl[:, tb : tb + 1],
                        scale=nhl[:, tb : tb + 1],
                    )
                    # d1 = (raw + 1) * q  (= 2*sim01*q)
                    nc.vector.scalar_tensor_tensor(
                        out=d1[:, w0 : w0 + wh],
                        in0=raw_ps[:, :wh],
                        scalar=1.0,
                        in1=q[:, w0 : w0 + wh],
                        op0=ADD,
                        op1=MULT,
                    )
                _eng(nc, D1Y_ENGINE).tensor_tensor(
                    out=d1y[:, :W], in0=d1[:, :W], in1=ybc_sb[:, :W], op=MULT
                )
                sw = pmain.tile([P, T + 1], F32, tag="sw")
                sy = pmain.tile([P, T + 1], F32, tag="sy")
                nc.vector.memset(sw[:, 0:1], 0.0)
                _eng(nc, SCAN_Y_ENGINE).memset(sy[:, 0:1], 0.0)
                nc.vector.tensor_tensor_scan(
                    out=sw[:, 1 : W + 1],
                    data0=q[:, :W],
                    data1=d1[:, :W],
                    initial=0.0,
                    op0=MULT,
                    op1=ADD,
                )
                _eng(nc, SCAN_Y_ENGINE).tensor_tensor_scan(
                    out=sy[:, 1 : W + 1],
                    data0=q[:, :W],
                    data1=d1y[:, :W],
                    initial=0.0,
                    op0=MULT,
                    op1=ADD,
                )
                # wsum[t] = S[t-1] read via diagonal: col (tb*128+p) of padded S
                junk = pmain.tile([P, P], F32, tag="junk")
                junk2 = pmain.tile([P, P], F32, tag="junk2")
                _eng(nc, EXTRACT_ENGINE).scalar_tensor_tensor(
                    out=junk[:],
                    in0=sw[:, tb * P : tb * P + P],
                    scalar=1.0,
                    in1=diag_sb[:],
                    op0=MULT,
                    op1=MULT,
                    accum_out=wsum[:, tb : tb + 1],
                )
                _eng(nc, EXTRACT_ENGINE).scalar_tensor_tensor(
                    out=junk2[:],
                    in0=sy[:, tb * P : tb * P + P],
                    scalar=1.0,
                    in1=diag_sb[:],
                    op0=MULT,
                    op1=MULT,
                    accum_out=ynum[:, tb : tb + 1],
                )

            # ---- finalize: yhat = clip(ynum / (wsum + 2e-6), .01, .99) ----
            wse = psmall.tile([P, NJ], F32, tag="wse")
            nc.vector.tensor_scalar(
                out=wse[:], in0=wsum[:], scalar1=2e-6, scalar2=None, op0=ADD
            )
            rcp = psmall.tile([P, NJ], F32, tag="rcp")
            nc.vector.reciprocal(out=rcp[:], in_=wse[:])
            yh = psmall.tile([P, NJ], F32, tag="yh")
            nc.vector.tensor_tensor(out=yh[:], in0=ynum[:], in1=rcp[:], op=MULT)
            yc = psmall.tile([P, NJ], F32, tag="yc")
            nc.vector.tensor_scalar(
                out=yc[:], in0=yh[:], scalar1=0.01, scalar2=0.99, op0=MAX, op1=MIN
            )
            nc.sync.dma_start(out=out[b], in_=yc[:])


def shard_inputs(y, problem_seq, embd_weight, lam_w, lam_b):
    """Build per-core input maps."""
    B = y.shape[0]
    assert B == N_CORES * NB_PER_CORE
    seq = np.ascontiguousarray(problem_seq).astype(np.int32)
    yf = np.ascontiguousarray(y).astype(np.float32)
    table = np.ascontiguousarray(embd_weight).astype(np.float32)
    lamw = np.ascontiguousarray(lam_w).reshape(P, 1).astype(np.float32)
    lamb = np.full((P, 1), np.float32(np.asarray(lam_b).reshape(-1)[0]))
    diag = np.eye(P, dtype=np.float32)

    in_maps = []
    for c in range(N_CORES):
        sl = slice(c * NB_PER_CORE, (c + 1) * NB_PER_CORE)
        # idx[b, p, j] = seq[b, j*128 + p]
        idx = seq[sl].reshape(NB_PER_CORE, NJ, P).transpose(0, 2, 1)
        ybc = np.broadcast_to(yf[sl][:, None, :], (NB_PER_CORE, P, T))
        in_maps.append(
            {
                "table": table,
                "idx": np.ascontiguousarray(idx),
                "ybc": np.ascontiguousarray(ybc),
                "lamw": lamw,
                "lamb": lamb,
                "diag": diag,
            }
        )
    return in_maps


def unshard_output(results):
    """results: list of 8 dicts with 'out' [4, 128, 8] -> yhat [32, 1024]."""
    parts = []
    for c in range(N_CORES):
        o = results[c]["out"]  # [NB, P, NJ]; yhat[b, j*128+p] = o[b, p, j]
        parts.append(o.transpose(0, 2, 1).reshape(NB_PER_CORE, T))
    return np.concatenate(parts, axis=0).astype(np.float32)


_NC_CACHE = None


def _get_program():
    global _NC_CACHE
    if _NC_CACHE is None:
        _NC_CACHE = build_program()
    return _NC_CACHE


def kernel(y, problem_seq, embd_weight, lam_w, lam_b, _trace=False, **trace_kwargs):
    nc = _get_program()
    in_maps = shard_inputs(y, problem_seq, embd_weight, lam_w, lam_b)
    res = run_bass_kernel_spmd(
        nc, in_maps, core_ids=list(range(N_CORES)), trace=_trace, **trace_kwargs
    )
    outp = unshard_output(res.results)
    if _trace:
        return outp, res
    return outp


if __name__ == "__main__":
    rng = np.random.default_rng(0)
    y = rng.random((32, T), dtype=np.float32)
    seq = rng.integers(0, N_VOCAB, size=(32, T)).astype(np.int32)
    emb = rng.standard_normal((N_VOCAB, P), dtype=np.float32)
    lw = (rng.standard_normal((P, 1), dtype=np.float32) / np.sqrt(P)).astype(np.float32)
    lb = (rng.standard_normal((1,), dtype=np.float32) * 0.01).astype(np.float32)
    outp = kernel(y, seq, emb, lw, lb)
    print("out", outp.shape, outp.dtype, outp[:2, :5])


# revision 9
# speedup vs baseline: 1.1222x; 1.1222x over previous
"""Bass/Trainium2 kernel for nn_ExpMovAvgModel (sparse_attention).

Math (per batch row b, query t, key s, H=128 hidden):
    x      = embd[seq]                        # [T, H] gathered rows
    xhat   = x / |x|                          # row-normalized
    raw    = xhat @ xhat.T                    # cosine similarity [T, T]
    sim01  = 0.5*(raw+1) masked to s < t
    delta  = reversed-cumsum_s(sim01)         # = sum_{v=s}^{t-1} sim01[v]
    lam    = exp(x @ lam_w + lam_b)
    w      = sim01 * exp(-lam*delta)
    yhat   = clip((w @ y) / (sum_s w + 1e-6), 0.01, 0.99)

Key restructure used here: with q[u] = exp(-lam*sim01[u]) and
d1[u] = (raw[u]+1)*q[u] = 2*sim01[u]*q[u], the forward scan
    S[s] = q[s]*S[s-1] + d1[s]
satisfies S[t-1] = 2*sum_u sim01[u]*exp(-lam*sum_{v=u}^{t-1} sim01[v])
        = 2*sum_s w[t,s].
So one tensor_tensor_scan per row-block + a diagonal extraction replaces
the masked reversed cumsum, the exp over [T,T], the weight multiply and
the row reduction.  A second scan with d1y = d1*y gives 2*(w @ y).
Causality comes for free: positions u >= t never enter S[t-1].

Sharding: data-parallel over batch B=32 -> 4 batches per core x 8 cores.
"""

import os
import sys

import numpy as np

for _p in ("/opt/trn_rl_repo",):
    if _p not in sys.path and os.path.isdir(_p):
        sys.path.append(_p)

import concourse.bass as bass
import concourse.tile as tile
from concourse import bacc, mybir
from concourse.bass_utils import run_bass_kernel_spmd

P = 128            # partitions / hidden dim
T = 1024           # sequence length
NJ = T // P        # 8 column-blocks
NB_PER_CORE = 4    # batches per core
N_CORES = 8
N_VOCAB = 50000

F32 = mybir.dt.float32
I32 = mybir.dt.int32

# engine knobs (tuned against the profile)
D1Y_ENGINE = "gpsimd"     # rawp1y = rawp1 * y elementwise multiply
EXTRACT_ENGINE = "vector" # diagonal extraction of the scan outputs (Pool lacks TensorScalarPtr)
SCAN_Y_ENGINE = "vector"  # second scan
MM_DTYPE = mybir.dt.float32r  # fp32r: 1 cycle/row at N>=256 vs 4 for fp32


def _eng(nc, name):
    return getattr(nc, name)


def build_program():
    nc = bacc.Bacc(
        "TRN2",
        target_bir_lowering=False,
        debug=False,
        num_devices=N_CORES,
    )

    table = nc.dram_tensor("table", [N_VOCAB, P], F32, kind="ExternalInput").ap()
    idx = nc.dram_tensor("idx", [NB_PER_CORE, P, NJ], I32, kind="ExternalInput").ap()
    ybc = nc.dram_tensor("ybc", [NB_PER_CORE, P, T], F32, kind="ExternalInput").ap()
    lamw = nc.dram_tensor("lamw", [P, 1], F32, kind="ExternalInput").ap()
    lamb = nc.dram_tensor("lamb", [P, 1], F32, kind="ExternalInput").ap()
    diag = nc.dram_tensor("diag", [P, P], F32, kind="ExternalInput").ap()
    out = nc.dram_tensor("out", [NB_PER_CORE, P, NJ], F32, kind="ExternalOutput").ap()

    with tile.TileContext(nc) as tc:
        _build_body(tc, table, idx, ybc, lamw, lamb, diag, out)

    nc.compile()
    return nc


def _build_body(tc, table, idx, ybc, lamw, lamb, diag, out):
    from contextlib import ExitStack

    nc = tc.nc
    Exp = mybir.ActivationFunctionType.Exp
    Sqrt = mybir.ActivationFunctionType.Sqrt
    ADD = mybir.AluOpType.add
    MULT = mybir.AluOpType.mult
    MAX = mybir.AluOpType.max
    MIN = mybir.AluOpType.min

    with ExitStack() as ctx:
        pconst = ctx.enter_context(tc.tile_pool(name="pconst", bufs=1))
        pbatch = ctx.enter_context(tc.tile_pool(name="pbatch", bufs=2))
        psmall = ctx.enter_context(tc.tile_pool(name="psmall", bufs=2))
        pmain = ctx.enter_context(tc.tile_pool(name="pmain", bufs=2))
        pps = ctx.enter_context(tc.tile_pool(name="pps", bufs=4, space="PSUM"))
        ppsx = ctx.enter_context(tc.tile_pool(name="ppsx", bufs=2, space="PSUM"))
        ppsl = ctx.enter_context(tc.tile_pool(name="ppsl", bufs=1, space="PSUM"))

        diag_sb = pconst.tile([P, P], F32)
        nc.sync.dma_start(out=diag_sb[:], in_=diag)
        lamw_sb = pconst.tile([P, 1], F32)
        nc.sync.dma_start(out=lamw_sb[:], in_=lamw)
        lamb_sb = pconst.tile([P, 1], F32)
        nc.sync.dma_start(out=lamb_sb[:], in_=lamb)

        for b in range(NB_PER_CORE):
            # ---- gather x rows: xg_j[p, :] = table[seq[j*128+p]] ----
            # HW indirect DMA needs one index per partition and a
            # contiguous output tile, so gather per column-block.
            idx_sb = psmall.tile([P, NJ], I32, tag="idx_sb")
            nc.sync.dma_start(out=idx_sb[:], in_=idx[b])
            xgs = []
            for j in range(NJ):
                xg = pbatch.tile([P, P], F32, tag=f"xg{j}")
                nc.gpsimd.indirect_dma_start(
                    out=xg[:],
                    out_offset=None,
                    in_=table,
                    in_offset=bass.IndirectOffsetOnAxis(
                        ap=idx_sb[:, j : j + 1], axis=0
                    ),
                )
                xgs.append(xg)

            # ---- row norms -> mag, rmag; normalize x ----
            magsq = psmall.tile([P, NJ], F32, tag="magsq")
            for j in range(NJ):
                sqjunk = pmain.tile([P, P], F32, tag="sqjunk")
                nc.scalar.activation(
                    out=sqjunk[:],
                    in_=xgs[j][:],
                    func=mybir.ActivationFunctionType.Square,
                    accum_out=magsq[:, j : j + 1],
                )
            mag = psmall.tile([P, NJ], F32, tag="mag")
            nc.scalar.activation(out=mag[:], in_=magsq[:], func=Sqrt)
            rmag = psmall.tile([P, NJ], F32, tag="rmag")
            nc.vector.reciprocal(out=rmag[:], in_=mag[:])
            xhat = pbatch.tile([P, NJ, P], F32, tag="xhat")
            for j in range(NJ):
                nc.vector.tensor_scalar(
                    out=xhat[:, j, :],
                    in0=xgs[j][:],
                    scalar1=rmag[:, j : j + 1],
                    scalar2=None,
                    op0=MULT,
                )

            # ---- transpose xhat -> xhatT [h, t] via PE ----
            # stored as float32r (rounded by the ACT copy) so the sim
            # matmuls run at 1 cycle/row instead of fp32's 4
            xhatT = pbatch.tile([P, T], MM_DTYPE, tag="xhatT")
            for half in range(2):
                xt_ps = ppsx.tile([P, 512], F32, tag="xt_ps")
                for k in range(4):
                    j = half * 4 + k
                    nc.tensor.transpose(
                        out=xt_ps[:, k * P : (k + 1) * P],
                        in_=xhat[:, j, :],
                        identity=diag_sb[:],
                    )
                nc.scalar.copy(
                    out=xhatT[:, half * 512 : (half + 1) * 512], in_=xt_ps[:]
                )


            # ---- lam = exp(mag * (xhat . lam_w) + lam_b); nhl = -lam/2 ----
            lamdot_ps = ppsl.tile([P, NJ], F32, tag="lamdot_ps")
            for j in range(NJ):
                nc.tensor.matmul(
                    out=lamdot_ps[:, j : j + 1],
                    lhsT=xhatT[:, j * P : (j + 1) * P].bitcast(F32),
                    rhs=lamw_sb[:],
                    start=True,
                    stop=True,
                )
            dotm = psmall.tile([P, NJ], F32, tag="dotm")
            nc.vector.tensor_tensor(
                out=dotm[:], in0=lamdot_ps[:], in1=mag[:], op=MULT
            )
            lam = psmall.tile([P, NJ], F32, tag="lam")
            nc.scalar.activation(
                out=lam[:], in_=dotm[:], func=Exp, bias=lamb_sb[:], scale=1.0
            )
            nhl = psmall.tile([P, NJ], F32, tag="nhl")
            nc.vector.tensor_scalar(
                out=nhl[:], in0=lam[:], scalar1=-0.5, scalar2=None, op0=MULT
            )

            # ---- y broadcast rows ----
            ybc_sb = pbatch.tile([P, T], F32, tag="ybc_sb")
            nc.sync.dma_start(out=ybc_sb[:], in_=ybc[b])

            wsum = psmall.tile([P, NJ], F32, tag="wsum")
            ynum = psmall.tile([P, NJ], F32, tag="ynum")

            # ---- main loop over query blocks ----
            for tb in range(NJ):
                W = (tb + 1) * P
                nhalf = 1 if W <= 512 else 2
                q = pmain.tile([P, T], F32, tag="q")
                rawp1 = pmain.tile([P, T], F32, tag="rawp1")
                rawp1y = pmain.tile([P, T], F32, tag="rawp1y")
                for h in range(nhalf):
                    w0 = h * 512
                    wh = min(W, (h + 1) * 512) - w0
                    raw_ps = pps.tile([P, 512], F32, tag="raw_ps")
                    nc.tensor.matmul(
                        out=raw_ps[:, :wh],
                        lhsT=xhatT[:, tb * P : (tb + 1) * P],
                        rhs=xhatT[:, w0 : w0 + wh],
                        start=True,
                        stop=True,
                    )
                    # q = exp(-lam/2 * raw - lam/2)
                    nc.scalar.activation(
                        out=q[:, w0 : w0 + wh],
                        in_=raw_ps[:, :wh],
                        func=Exp,
                        bias=nhl[:, tb : tb + 1],
                        scale=nhl[:, tb : tb + 1],
                    )
                    # rawp1 = raw + 1  (= 2*sim01)
                    nc.scalar.activation(
                        out=rawp1[:, w0 : w0 + wh],
                        in_=raw_ps[:, :wh],
                        func=mybir.ActivationFunctionType.Identity,
                        bias=1.0,
                        scale=1.0,
                    )
                _eng(nc, D1Y_ENGINE).tensor_tensor(
                    out=rawp1y[:, :W], in0=rawp1[:, :W], in1=ybc_sb[:, :W], op=MULT
                )
                # scan: S[s] = (S[s-1] + rawp1[s]) * q[s]
                #  => S[t-1] = sum_u (raw[u]+1) prod_{v=u..t-1} q[v] = 2*wsum[t]
                sw = pmain.tile([P, T + 1], F32, tag="sw")
                sy = pmain.tile([P, T + 1], F32, tag="sy")
                nc.vector.memset(sw[:, 0:1], 0.0)
                _eng(nc, SCAN_Y_ENGINE).memset(sy[:, 0:1], 0.0)
                nc.vector.tensor_tensor_scan(
                    out=sw[:, 1 : W + 1],
                    data0=rawp1[:, :W],
                    data1=q[:, :W],
                    initial=0.0,
                    op0=ADD,
                    op1=MULT,
                )
                _eng(nc, SCAN_Y_ENGINE).tensor_tensor_scan(
                    out=sy[:, 1 : W + 1],
                    data0=rawp1y[:, :W],
                    data1=q[:, :W],
                    initial=0.0,
                    op0=ADD,
                    op1=MULT,
                )
                # wsum[t] = S[t-1] read via diagonal: col (tb*128+p) of padded S
                junk = pmain.tile([P, P], F32, tag="junk")
                junk2 = pmain.tile([P, P], F32, tag="junk2")
                _eng(nc, EXTRACT_ENGINE).scalar_tensor_tensor(
                    out=junk[:],
                    in0=sw[:, tb * P : tb * P + P],
                    scalar=1.0,
                    in1=diag_sb[:],
                    op0=MULT,
                    op1=MULT,
                    accum_out=wsum[:, tb : tb + 1],
                )
                _eng(nc, EXTRACT_ENGINE).scalar_tensor_tensor(
                    out=junk2[:],
                    in0=sy[:, tb * P : tb * P + P],
                    scalar=1.0,
                    in1=diag_sb[:],
                    op0=MULT,
                    op1=MULT,
                    accum_out=ynum[:, tb : tb + 1],
                )

            # ---- finalize: yhat = clip(ynum / (wsum + 2e-6), .01, .99) ----
            wse = psmall.tile([P, NJ], F32, tag="wse")
            nc.vector.tensor_scalar(
                out=wse[:], in0=wsum[:], scalar1=2e-6, scalar2=None, op0=ADD
            )
            rcp = psmall.tile([P, NJ], F32, tag="rcp")
            nc.vector.reciprocal(out=rcp[:], in_=wse[:])
            yh = psmall.tile([P, NJ], F32, tag="yh")
            nc.vector.tensor_tensor(out=yh[:], in0=ynum[:], in1=rcp[:], op=MULT)
            yc = psmall.tile([P, NJ], F32, tag="yc")
            nc.vector.tensor_scalar(
                out=yc[:], in0=yh[:], scalar1=0.01, scalar2=0.99, op0=MAX, op1=MIN
            )
            nc.sync.dma_start(out=out[b], in_=yc[:])


def shard_inputs(y, problem_seq, embd_weight, lam_w, lam_b):
    """Build per-core input maps."""
    B = y.shape[0]
    assert B == N_CORES * NB_PER_CORE
    seq = np.ascontiguousarray(problem_seq).astype(np.int32)
    yf = np.ascontiguousarray(y).astype(np.float32)
    table = np.ascontiguousarray(embd_weight).astype(np.float32)
    lamw = np.ascontiguousarray(lam_w).reshape(P, 1).astype(np.float32)
    lamb = np.full((P, 1), np.float32(np.asarray(lam_b).reshape(-1)[0]))
    diag = np.eye(P, dtype=np.float32)

    in_maps = []
    for c in range(N_CORES):
        sl = slice(c * NB_PER_CORE, (c + 1) * NB_PER_CORE)
        # idx[b, p, j] = seq[b, j*128 + p]
        idx = seq[sl].reshape(NB_PER_CORE, NJ, P).transpose(0, 2, 1)
        ybc = np.broadcast_to(yf[sl][:, None, :], (NB_PER_CORE, P, T))
        in_maps.append(
            {
                "table": table,
                "idx": np.ascontiguousarray(idx),
                "ybc": np.ascontiguousarray(ybc),
                "lamw": lamw,
                "lamb": lamb,
                "diag": diag,
            }
        )
    return in_maps


def unshard_output(results):
    """results: list of 8 dicts with 'out' [4, 128, 8] -> yhat [32, 1024]."""
    parts = []
    for c in range(N_CORES):
        o = results[c]["out"]  # [NB, P, NJ]; yhat[b, j*128+p] = o[b, p, j]
        parts.append(o.transpose(0, 2, 1).reshape(NB_PER_CORE, T))
    return np.concatenate(parts, axis=0).astype(np.float32)


_NC_CACHE = None


def _get_program():
    global _NC_CACHE
    if _NC_CACHE is None:
        _NC_CACHE = build_program()
    return _NC_CACHE


def kernel(y, problem_seq, embd_weight, lam_w, lam_b, _trace=False, **trace_kwargs):
    nc = _get_program()
    in_maps = shard_inputs(y, problem_seq, embd_weight, lam_w, lam_b)
    res = run_bass_kernel_spmd(
        nc, in_maps, core_ids=list(range(N_CORES)), trace=_trace, **trace_kwargs
    )
    outp = unshard_output(res.results)
    if _trace:
        return outp, res
    return outp


if __name__ == "__main__":
    rng = np.random.default_rng(0)
    y = rng.random((32, T), dtype=np.float32)
    seq = rng.integers(0, N_VOCAB, size=(32, T)).astype(np.int32)
    emb = rng.standard_normal((N_VOCAB, P), dtype=np.float32)
    lw = (rng.standard_normal((P, 1), dtype=np.float32) / np.sqrt(P)).astype(np.float32)
    lb = (rng.standard_normal((1,), dtype=np.float32) * 0.01).astype(np.float32)
    outp = kernel(y, seq, emb, lw, lb)
    print("out", outp.shape, outp.dtype, outp[:2, :5])
